# revision 24
# baseline (speedup 1.0000x reference)
# Multi-head attention (B=2, S=2048, D=1024, H=16) on 8 TRN2 NeuronCores.
#
# Sharding (hardcoded): core c in [0..8) handles batch b = c//4 and head
# group g = c%4 (4 heads = 256 output features of wq/wk/wv, 256 input rows
# of wo). Each core computes a partial output projection [S, D]; the host
# sums the 4 partials per batch and adds wo_bias (row-parallel unshard).
#
# Device-side schedule:
#   - DMA stream is ordered so the first projection matmul fires ~10us in
#     (bias rows first, then wq/xq0/wk/xk0/...); the scalar engine's exp
#     stream (the attention throughput floor at 1 elem/cycle/lane, ~143us
#     for 16.8M scores/core) starts ~35us in and never breaks: each
#     (q-chunk, head) stage pre-emits the next stage's first two score
#     tiles and hands its PV tail + softmax-normalize into the next
#     stage's k-tile slots;
#   - per stage, score matmuls run >=2 k-tiles ahead of the P@V matmuls
#     (s_ps double-buffered, 2+2+2+2 PSUM banks exactly);
#   - chunk-1 projection tiles and qc0 output projections ride as
#     per-slot PE filler inside later stages, placed by dependency;
#   - softmax denominator via a ones-column appended to each head's V;
#   - mask lives in SBUF one q-chunk at a time (re-DMA'd for qc1 right
#     after the last qc0 reader of each k-tile);
#   - biases enter as K=128 rank-1 matmuls against ones/128 (no 32-row
#     PE-mode switches in the projection stream).
import functools
import sys

import numpy as np

try:
    import concourse  # noqa: F401
except ImportError:  # harness env without the default path
    sys.path.insert(0, "/opt/trn_rl_repo")
    sys.path.insert(0, "/opt/pypackages")

import ml_dtypes

BF16 = ml_dtypes.bfloat16

B, S, D, H = 2, 2048, 1024, 16
HD = D // H          # 64
NCORES = 8
GH = 4               # head groups (tensor-parallel)
HPG = H // GH        # heads per group = 4
DG = D // GH         # features per group = 256
P = 128              # partitions
TDIN = D // P        # 8 din tiles
NCH = 2              # x-chunks of 1024 for projections
CW = S // NCH        # 1024
QC = 2               # q-chunks of 1024 for attention
QW = S // QC         # 1024
KT = S // P          # 16 k tiles
NT2 = DG // P        # 2 dout tiles per group


def build_graph():
    """Build the SPMD Bass graph (identical on all 8 cores)."""
    from contextlib import ExitStack

    from concourse import bacc, mybir, tile

    f32 = mybir.dt.float32
    bf16 = mybir.dt.bfloat16
    EXP = mybir.ActivationFunctionType.Exp

    nc = bacc.Bacc(
        "TRN2", target_bir_lowering=False, debug=False, num_devices=NCORES
    )

    xq = nc.dram_tensor("xq_t", (P, TDIN, S), bf16, kind="ExternalInput")
    xk = nc.dram_tensor("xk_t", (P, TDIN, S), bf16, kind="ExternalInput")
    xv = nc.dram_tensor("xv_t", (P, TDIN, S), bf16, kind="ExternalInput")
    mk = nc.dram_tensor("mask_t", (S, S), bf16, kind="ExternalInput")
    wq = nc.dram_tensor("wq", (P, TDIN, DG), bf16, kind="ExternalInput")
    wk = nc.dram_tensor("wk", (P, TDIN, DG), bf16, kind="ExternalInput")
    wv = nc.dram_tensor("wv", (P, TDIN, DG), bf16, kind="ExternalInput")
    # wo pre-arranged host-side to [64, HPG, D] (j, h, n) so each head's
    # 64 rows sit on partitions 0..63.
    wo = nc.dram_tensor("wo", (HD, HPG, D), bf16, kind="ExternalInput")
    qb = nc.dram_tensor("qb", (1, DG), bf16, kind="ExternalInput")
    kb = nc.dram_tensor("kb", (1, DG), bf16, kind="ExternalInput")
    vb = nc.dram_tensor("vb", (1, DG), bf16, kind="ExternalInput")
    out = nc.dram_tensor("out", (S, D), bf16, kind="ExternalOutput")

    with tile.TileContext(nc) as tc, ExitStack() as ctx:
        wpool = ctx.enter_context(tc.tile_pool(name="wpool", bufs=1))
        qkpool = ctx.enter_context(tc.tile_pool(name="qk", bufs=1))
        vpool = ctx.enter_context(tc.tile_pool(name="vsb", bufs=1))
        mpool = ctx.enter_context(tc.tile_pool(name="msk", bufs=1))
        xpool = ctx.enter_context(tc.tile_pool(name="xin", bufs=1))
        xvpool = ctx.enter_context(tc.tile_pool(name="xvin", bufs=1))
        ptpool = ctx.enter_context(tc.tile_pool(name="ptile", bufs=6))
        npool = ctx.enter_context(tc.tile_pool(name="norm", bufs=1))
        otnpool = ctx.enter_context(tc.tile_pool(name="otn", bufs=1))
        outpool = ctx.enter_context(tc.tile_pool(name="outsb", bufs=2))
        dpool = ctx.enter_context(tc.tile_pool(name="dscr", bufs=2, space="DRAM"))
        # PSUM: 2x2 banks score double-buffer + 2 banks PV accum + 2 banks
        # scratch (projections early / out-proj late) = 8 banks exactly.
        sps_pool = ctx.enter_context(tc.tile_pool(name="sps", bufs=2, space="PSUM"))
        ops_pool = ctx.enter_context(tc.tile_pool(name="ops", bufs=1, space="PSUM"))
        scr_pool = ctx.enter_context(tc.tile_pool(name="scrps", bufs=2, space="PSUM"))

        # ---- persistent SBUF tensors -------------------------------------
        # DMA order: bias rows (tiny, feed the first PE ops), then
        # wq, xq0, wk, xk0, wv, xv0, wo so projections start ~8us in.
        qb_row = wpool.tile([1, DG], bf16)
        kb_row = wpool.tile([1, DG], bf16)
        vb_row = wpool.tile([1, DG], bf16)
        nc.sync.dma_start(qb_row[:], qb.ap())
        nc.sync.dma_start(kb_row[:], kb.ap())
        nc.sync.dma_start(vb_row[:], vb.ap())
        ones_row = wpool.tile([1, P], bf16)
        nc.vector.memset(ones_row[:], 1.0)
        ones2 = wpool.tile([P, 512], bf16)
        nc.vector.memset(ones2[:], 1.0 / P)
        inv128 = wpool.tile([P, P], bf16)
        nc.vector.memset(inv128[:], 1.0 / P)
        qb_sb = wpool.tile([P, DG], bf16)
        kb_sb = wpool.tile([P, DG], bf16)
        vb_sb = wpool.tile([P, DG], bf16)
        for row_, bc_, nm_ in (
            (qb_row, qb_sb, "q"), (kb_row, kb_sb, "k"), (vb_row, vb_sb, "v")
        ):
            bps = scr_pool.tile([P, 512], f32, tag="ps", name=f"bc_{nm_}")
            nc.tensor.matmul(
                bps[:, 0:DG], lhsT=ones_row[:], rhs=row_[:],
                start=True, stop=True,
            )
            nc.scalar.copy(bc_[:], bps[:, 0:DG])
        wq_sb = wpool.tile([P, TDIN, DG], bf16)
        wk_sb = wpool.tile([P, TDIN, DG], bf16)
        wv_sb = wpool.tile([P, TDIN, DG], bf16)
        x0 = []

        def _emit_x_one(xdram, tag, c):
            pool_ = xvpool if tag == "xv" else xpool
            t_ = pool_.tile([P, TDIN, CW], bf16, tag=tag)
            for th_ in range(4):
                nc.sync.dma_start(
                    t_[:, th_ * 2 : (th_ + 1) * 2, :],
                    xdram.ap()[
                        :, th_ * 2 : (th_ + 1) * 2, c * CW : (c + 1) * CW
                    ],
                )
            return t_

        for wsb_, wdr_, xdr_, xtag_ in (
            (wq_sb, wq, xq, "xq"),
            (wk_sb, wk, xk, "xk"),
            (wv_sb, wv, xv, "xv"),
        ):
            for th_ in range(2):
                nc.sync.dma_start(
                    wsb_[:, th_ * 4 : (th_ + 1) * 4, :],
                    wdr_.ap()[:, th_ * 4 : (th_ + 1) * 4, :],
                )
            x0.append(_emit_x_one(xdr_, xtag_, 0))
        wo_sb = wpool.tile([HD, HPG, D], bf16)
        nc.sync.dma_start(wo_sb[:], wo.ap())
        qT_sb = qkpool.tile([P, NT2, S], bf16)   # q projection, transposed
        kT_sb = qkpool.tile([P, NT2, S], bf16)
        # partition-swapped copies (rows 0:64 <-> 64:128) so each head's
        # two score halves can stream on BOTH 64x128 PE row tiles at once
        qTd_sb = qkpool.tile([P, NT2, S], bf16)
        kTd_sb = qkpool.tile([P, NT2, S], bf16)
        # v blocks: per k-tile, per head: [v(64) | ones] -> 65 cols
        v_sb = vpool.tile([P, KT, HPG * (HD + 1)], bf16)
        nc.vector.memset(
            v_sb[:].rearrange("p s (h x) -> p s h x", h=HPG)[:, :, :, HD : HD + 1],
            1.0,
        )
        mask_sb = mpool.tile([P, KT, QW], bf16)
        mk_r = mk.ap().rearrange("(t p) q -> p t q", p=P)

        # warm the exp table set while DMAs stream (off critical path)
        warm = npool.tile([1, 32], bf16, tag="warm")
        nc.vector.memset(warm[:], 0.0)
        nc.scalar.activation(warm[:], warm[:], EXP)

        otn_sb = otnpool.tile([HD, HPG, S], bf16)

        # ---- emit helpers ------------------------------------------------
        def emit_x_dma(c):
            return [
                _emit_x_one(xdram, tag, c)
                for xdram, tag in ((xq, "xq"), (xk, "xk"), (xv, "xv"))
            ]

        def emit_mask_dma(kts, qc):
            for kt in kts:
                nc.sync.dma_start(
                    mask_sb[:, kt, :],
                    mk_r[:, kt, qc * QW : (qc + 1) * QW],
                )

        def emit_qk_tile(c, xch, wsb, bias_sb, dest, half, dt):
            s0 = c * CW + half * 512
            ps = scr_pool.tile([P, 512], f32, tag="ps", name=f"pj{c}{half}{dt}")
            for ktl in range(TDIN):
                nc.tensor.matmul(
                    ps[:],
                    lhsT=wsb[:, ktl, dt * P : (dt + 1) * P],
                    rhs=xch[:, ktl, half * 512 : (half + 1) * 512],
                    start=(ktl == 0),
                    stop=False,
                )
            nc.tensor.matmul(
                ps[:],
                lhsT=bias_sb[:, dt * P : (dt + 1) * P],
                rhs=ones2[:],
                start=False,
                stop=True,
            )
            nc.vector.tensor_copy(dest[:, dt, s0 : s0 + 512], ps[:])
            dup = qTd_sb if dest is qT_sb else kTd_sb
            nc.sync.dma_start(
                dup[HD:P, dt, s0 : s0 + 512], dest[0:HD, dt, s0 : s0 + 512]
            )
            nc.sync.dma_start(
                dup[0:HD, dt, s0 : s0 + 512], dest[HD:P, dt, s0 : s0 + 512]
            )

        def emit_v_tile(c, xv_c, m):
            st = c * (CW // P) + m
            ps = scr_pool.tile([P, 512], f32, tag="ps", name=f"pv_{c}_{m}")
            for ktl in range(TDIN):
                nc.tensor.matmul(
                    ps[:, 0:DG],
                    lhsT=xv_c[:, ktl, m * P : (m + 1) * P],
                    rhs=wv_sb[:, ktl, :],
                    start=(ktl == 0),
                    stop=False,
                )
            nc.tensor.matmul(
                ps[:, 0:DG],
                lhsT=inv128[:],
                rhs=vb_sb[:],
                start=False,
                stop=True,
            )
            nc.vector.tensor_copy(
                v_sb[:, st, :].rearrange("p (h x) -> p h x", h=HPG)[:, :, 0:HD],
                ps[:, 0:DG].rearrange("p (h x) -> p h x", h=HPG),
            )

        def emit_proj_chunk(c, xq_c, xk_c, xv_c):
            for xch, wsb, bias_sb, dest in (
                (xq_c, wq_sb, qb_sb, qT_sb),
                (xk_c, wk_sb, kb_sb, kT_sb),
            ):
                for half in range(2):
                    for dt in range(NT2):
                        emit_qk_tile(c, xch, wsb, bias_sb, dest, half, dt)
            for m in range(CW // P):
                emit_v_tile(c, xv_c, m)

        # out-projection for one (st, nch) quarter; 4 accumulating matmuls
        osb2_live = {}

        def emit_outproj_part(st, nch):
            if st not in osb2_live:
                osb2_live[st] = outpool.tile(
                    [P, D], bf16, tag="outsb", name=f"outsb_{st}"
                )
            osb2 = osb2_live[st]
            op_ps = scr_pool.tile([P, 512], f32, tag="ps", name=f"op_{st}_{nch}")
            for h_ in range(HPG):
                nc.tensor.matmul(
                    op_ps[:],
                    lhsT=otn_sb[:, h_, st * P : (st + 1) * P],
                    rhs=wo_sb[:, h_, nch * 512 : (nch + 1) * 512],
                    start=(h_ == 0),
                    stop=(h_ == HPG - 1),
                )
            nc.vector.tensor_copy(osb2[:, nch * 512 : (nch + 1) * 512], op_ps[:])
            if nch == 1:
                nc.sync.dma_start(out.ap()[st * P : (st + 1) * P, :], osb2[:])
                del osb2_live[st]

        # ---- attention ---------------------------------------------------
        def make_head(qc, h):
            """Returns (sc, pv, norm) emitters for one (qc, h)."""
            t, po = h // 2, (h % 2) * HD
            o_ps = ops_pool.tile([HD + 1, QW], f32, tag="ops", name=f"o_{qc}_{h}")
            pts = {}

            def sc(kt, post_mask=None):
                s_ps = sps_pool.tile(
                    [P, QW], f32, tag="sps", name=f"s_{qc}_{h}_{kt}"
                )
                po2 = HD - po  # opposite partition half (swapped copies)
                nc.tensor.matmul(
                    s_ps[:, 0:512],
                    lhsT=kT_sb[po : po + HD, t, kt * P : (kt + 1) * P],
                    rhs=qT_sb[po : po + HD, t, qc * QW : qc * QW + 512],
                    start=True,
                    stop=True,
                )
                nc.tensor.matmul(
                    s_ps[:, 512:1024],
                    lhsT=kTd_sb[po2 : po2 + HD, t, kt * P : (kt + 1) * P],
                    rhs=qTd_sb[
                        po2 : po2 + HD, t, qc * QW + 512 : qc * QW + 1024
                    ],
                    start=True,
                    stop=True,
                )
                pt = ptpool.tile([P, QW], bf16, tag="p", name=f"p_{qc}_{h}_{kt}")
                nc.scalar.activation(pt[:], s_ps[:], EXP, scale=0.125)
                nc.vector.tensor_mul(pt[:], pt[:], mask_sb[:, kt, :])
                if post_mask is not None:
                    post_mask(kt)
                pts[kt] = pt

            def pv(kt):
                pt = pts.pop(kt)
                for hf in range(2):
                    nc.tensor.matmul(
                        o_ps[:, hf * 512 : (hf + 1) * 512],
                        lhsT=v_sb[:, kt, h * 65 : (h + 1) * 65],
                        rhs=pt[:, hf * 512 : (hf + 1) * 512],
                        start=(kt == 0),
                        stop=(kt == KT - 1),
                    )

            def norm():
                # baseline normalize: approx-recip of the denominator row,
                # DRAM-bounce broadcast, one TT multiply.
                rec65 = npool.tile([HD + 1, QW], f32, tag="rec")
                nc.vector.reciprocal_approx_fast(out=rec65[:], in_=o_ps[:])
                osb = npool.tile([HD, QW], f32, tag="osb")
                nc.vector.tensor_copy(osb[:], o_ps[0:HD, :])
                scr = dpool.tile([1, QW], f32, tag="scr", name=f"sc_{qc}_{h}")
                nc.sync.dma_start(scr[:], rec65[HD : HD + 1, :])
                rb = npool.tile([HD, QW], f32, tag="rb")
                nc.sync.dma_start(rb[:], scr[:].to_broadcast((HD, QW)))
                nc.vector.tensor_mul(
                    otn_sb[:, h, qc * QW : (qc + 1) * QW], osb[:], rb[:]
                )

            return sc, pv, norm

        # ---- flat scheduler: 8 (qc, h) stages -------------------------
        # Stage i's PV tail + norm are carried into stage i+1's first
        # slots, and stage i pre-emits stage i+1's first two score tiles,
        # so the scalar exp stream never breaks at boundaries. Projection
        # chunk-1 pieces and qc0 out-projections ride along as per-slot
        # PE filler where their dependencies allow.
        def stage_flow(sc, pv, norm, start_kt, queue, post_mask, nxt,
                       min_fill_kt=2):
            state = {"pv": 0}

            def drain_pv(upto):
                while state["pv"] <= min(upto, KT - 3):
                    pv(state["pv"])
                    state["pv"] += 1

            for kt in range(start_kt, KT):
                sc(kt, post_mask)
                if queue and kt >= min_fill_kt:
                    queue.pop(0)()
                if kt >= 5:
                    drain_pv(kt - 2)
                if nxt is not None and kt >= KT - 2:
                    nxt[0](kt - (KT - 2), nxt[1])
            drain_pv(KT - 3)
            return [lambda: pv(KT - 2), lambda: pv(KT - 1), norm]

        # ---- main emission ----------------------------------------------
        emit_mask_dma(range(0, 8), 0)
        xk1 = _emit_x_one(xk, "xk", 1)
        xv1 = _emit_x_one(xv, "xv", 1)
        emit_mask_dma(range(8, KT), 0)
        xq1 = _emit_x_one(xq, "xq", 1)
        emit_proj_chunk(0, *x0)

        stages = [make_head(q_, h_) for q_ in range(QC) for h_ in range(HPG)]
        post_masks = [None] * 8

        def mask_qc1_hook(kt):
            nc.sync.dma_start(mask_sb[:, kt, :], mk_r[:, kt, QW : 2 * QW])

        post_masks[HPG - 1] = mask_qc1_hook

        # filler queues (consumed one per k-tile slot)
        fq = [[] for _ in range(8)]
        # chunk-1 k (dt0 before sc(8)/sc(12) of heads 0/1) and v (before
        # this head's own PVs reach k-tile 8) ride in stage 0
        fq[0] = [
            lambda: emit_qk_tile(1, xk1, wk_sb, kb_sb, kT_sb, 0, 0),
            lambda: emit_qk_tile(1, xk1, wk_sb, kb_sb, kT_sb, 1, 0),
        ] + [(lambda m_=m: emit_v_tile(1, xv1, m_)) for m in range(8)]
        # k dt1 (heads 2/3) + all of q chunk 1 (needed at qc1) in stage 1
        fq[1] = [
            lambda: emit_qk_tile(1, xk1, wk_sb, kb_sb, kT_sb, 0, 1),
            lambda: emit_qk_tile(1, xk1, wk_sb, kb_sb, kT_sb, 1, 1),
            lambda: emit_qk_tile(1, xq1, wq_sb, qb_sb, qT_sb, 0, 0),
            lambda: emit_qk_tile(1, xq1, wq_sb, qb_sb, qT_sb, 1, 0),
        ]
        fq[2] = [lambda: emit_qk_tile(1, xq1, wq_sb, qb_sb, qT_sb, 0, 1)]
        fq[3] = [lambda: emit_qk_tile(1, xq1, wq_sb, qb_sb, qT_sb, 1, 1)]
        # qc0 out-projection fillers during qc1 stages
        opj = [(st, nch) for st in range(S // P // 2) for nch in range(2)]
        for h in range(HPG):
            fq[4 + h] = [
                (lambda s=st, n=nch: emit_outproj_part(s, n))
                for st, nch in opj[h * 4 : (h + 1) * 4]
            ]

        carry = []
        for i in range(8):
            sc, pv, norm = stages[i]
            nxt = (stages[i + 1][0], post_masks[i + 1]) if i < 7 else None
            carry = stage_flow(
                sc, pv, norm,
                start_kt=0 if i == 0 else 2,
                queue=carry + fq[i],
                post_mask=post_masks[i],
                nxt=nxt,
                min_fill_kt=4 if i == 0 else 2,
            )
        for fn in carry:
            fn()

        # qc1 out-proj tail: two PSUM pipelines (scratch + freed score
        # banks) so the 16 remaining parts drain twice as wide.
        def emit_outproj_sps(st):
            op = sps_pool.tile([P, QW], f32, tag="sps", name=f"opx_{st}")
            for nch in range(2):
                for h_ in range(HPG):
                    nc.tensor.matmul(
                        op[:, nch * 512 : (nch + 1) * 512],
                        lhsT=otn_sb[:, h_, st * P : (st + 1) * P],
                        rhs=wo_sb[:, h_, nch * 512 : (nch + 1) * 512],
                        start=(h_ == 0),
                        stop=(h_ == HPG - 1),
                    )
            osb2 = outpool.tile([P, D], bf16, tag="outsb", name=f"ox_{st}")
            nc.vector.tensor_copy(osb2[:], op[:])
            nc.sync.dma_start(out.ap()[st * P : (st + 1) * P, :], osb2[:])

        for st in range(S // P // 2, S // P):
            if st % 2 == 0:
                emit_outproj_sps(st)
            else:
                emit_outproj_part(st, 0)
                emit_outproj_part(st, 1)

    nc.compile()
    return nc


@functools.lru_cache(maxsize=1)
def _graph():
    return build_graph()


def make_in_maps(
    query, key, value, mask,
    wq_kernel, wq_bias, wk_kernel, wk_bias,
    wv_kernel, wv_bias, wo_kernel, wo_bias,
):
    q = np.asarray(query, np.float32)
    k = np.asarray(key, np.float32)
    v = np.asarray(value, np.float32)
    mask = np.asarray(mask)
    wqk = np.asarray(wq_kernel, np.float32)
    wkk = np.asarray(wk_kernel, np.float32)
    wvk = np.asarray(wv_kernel, np.float32)
    wok = np.asarray(wo_kernel, np.float32)

    def tile_x(a):  # [S, D] -> [P, TDIN, S] pre-tiled transpose
        return np.ascontiguousarray(
            a.T.reshape(TDIN, P, S).transpose(1, 0, 2)
        ).astype(BF16)

    xt = [[tile_x(x[b]) for x in (q, k, v)] for b in range(B)]
    mt = [
        np.ascontiguousarray(mask[b].T.astype(np.float32)).astype(BF16)
        for b in range(B)
    ]
    in_maps = []
    for c in range(NCORES):
        b, g = divmod(c, GH)
        cs = slice(g * DG, (g + 1) * DG)
        wo_arr = np.ascontiguousarray(
            wok[cs, :].reshape(HPG, HD, D).transpose(1, 0, 2)
        ).astype(BF16)
        in_maps.append(
            {
                "xq_t": xt[b][0],
                "xk_t": xt[b][1],
                "xv_t": xt[b][2],
                "mask_t": mt[b],
                "wq": np.ascontiguousarray(wqk[:, cs].reshape(TDIN, P, DG).transpose(1, 0, 2)).astype(BF16),
                "wk": np.ascontiguousarray(wkk[:, cs].reshape(TDIN, P, DG).transpose(1, 0, 2)).astype(BF16),
                "wv": np.ascontiguousarray(wvk[:, cs].reshape(TDIN, P, DG).transpose(1, 0, 2)).astype(BF16),
                "wo": wo_arr,
                "qb": np.asarray(wq_bias, np.float32)[cs].reshape(1, DG).astype(BF16),
                "kb": np.asarray(wk_bias, np.float32)[cs].reshape(1, DG).astype(BF16),
                "vb": np.asarray(wv_bias, np.float32)[cs].reshape(1, DG).astype(BF16),
            }
        )
    return in_maps


def combine_outputs(results, wo_bias):
    outs = np.stack([np.asarray(r["out"], np.float32) for r in results])
    full = outs.reshape(B, GH, S, D).sum(axis=1)
    return (full + np.asarray(wo_bias, np.float32)[None, None, :]).astype(
        np.float32
    )


def kernel(**inputs):
    from concourse import bass_utils

    nc = _graph()
    in_maps = make_in_maps(**inputs)
    res = bass_utils.run_bass_kernel_spmd(
        nc, in_maps, core_ids=list(range(NCORES))
    )
    return combine_outputs(res.results, inputs["wo_bias"])


# revision 25
# speedup vs baseline: 1.1998x; 1.1998x over previous
# Multi-head attention (B=2, S=2048, D=1024, H=16) on 8 TRN2 NeuronCores.
#
# Sharding (hardcoded): core c in [0..8) handles batch b = c//4 and head
# group g = c%4 (4 heads = 256 output features of wq/wk/wv, 256 input rows
# of wo). Each core computes a partial output projection [S, D]; the host
# sums the 4 partials per batch and adds wo_bias (row-parallel unshard).
#
# Device-side schedule:
#   - DMA stream is ordered so the first projection matmul fires ~10us in
#     (bias rows first, then wq/xq0/wk/xk0/...); the scalar engine's exp
#     stream (the attention throughput floor at 1 elem/cycle/lane, ~143us
#     for 16.8M scores/core) starts ~35us in and never breaks: each
#     (q-chunk, head) stage pre-emits the next stage's first two score
#     tiles and hands its PV tail + softmax-normalize into the next
#     stage's k-tile slots;
#   - per stage, score matmuls run >=2 k-tiles ahead of the P@V matmuls
#     (s_ps double-buffered, 2+2+2+2 PSUM banks exactly);
#   - chunk-1 projection tiles and qc0 output projections ride as
#     per-slot PE filler inside later stages, placed by dependency;
#   - softmax denominator via a ones-column appended to each head's V;
#   - mask lives in SBUF one q-chunk at a time (re-DMA'd for qc1 right
#     after the last qc0 reader of each k-tile);
#   - biases enter as K=128 rank-1 matmuls against ones/128 (no 32-row
#     PE-mode switches in the projection stream).
import functools
import sys

import numpy as np

try:
    import concourse  # noqa: F401
except ImportError:  # harness env without the default path
    sys.path.insert(0, "/opt/trn_rl_repo")
    sys.path.insert(0, "/opt/pypackages")

import ml_dtypes

BF16 = ml_dtypes.bfloat16

B, S, D, H = 2, 2048, 1024, 16
HD = D // H          # 64
NCORES = 8
GH = 4               # head groups (tensor-parallel)
HPG = H // GH        # heads per group = 4
DG = D // GH         # features per group = 256
P = 128              # partitions
TDIN = D // P        # 8 din tiles
NCH = 2              # x-chunks of 1024 for projections
CW = S // NCH        # 1024
QC = 2               # q-chunks of 1024 for attention
QW = S // QC         # 1024
KT = S // P          # 16 k tiles
NT2 = DG // P        # 2 dout tiles per group


def build_graph():
    """Build the SPMD Bass graph (identical on all 8 cores)."""
    from contextlib import ExitStack

    from concourse import bacc, mybir, tile

    f32 = mybir.dt.float32
    bf16 = mybir.dt.bfloat16
    EXP = mybir.ActivationFunctionType.Exp

    nc = bacc.Bacc(
        "TRN2", target_bir_lowering=False, debug=False, num_devices=NCORES
    )

    xq = nc.dram_tensor("xq_t", (P, TDIN, S), bf16, kind="ExternalInput")
    xk = nc.dram_tensor("xk_t", (P, TDIN, S), bf16, kind="ExternalInput")
    xv = nc.dram_tensor("xv_t", (P, TDIN, S), bf16, kind="ExternalInput")
    mk = nc.dram_tensor("mask_t", (S, S), bf16, kind="ExternalInput")
    wq = nc.dram_tensor("wq", (P, TDIN, DG), bf16, kind="ExternalInput")
    wk = nc.dram_tensor("wk", (P, TDIN, DG), bf16, kind="ExternalInput")
    wv = nc.dram_tensor("wv", (P, TDIN, DG), bf16, kind="ExternalInput")
    # wo pre-arranged host-side to [64, HPG, D] (j, h, n) so each head's
    # 64 rows sit on partitions 0..63.
    wo = nc.dram_tensor("wo", (HD, HPG, D), bf16, kind="ExternalInput")
    qb = nc.dram_tensor("qb", (1, DG), bf16, kind="ExternalInput")
    kb = nc.dram_tensor("kb", (1, DG), bf16, kind="ExternalInput")
    vb = nc.dram_tensor("vb", (1, DG), bf16, kind="ExternalInput")
    out = nc.dram_tensor("out", (S, D), bf16, kind="ExternalOutput")

    with tile.TileContext(nc) as tc, ExitStack() as ctx:
        wpool = ctx.enter_context(tc.tile_pool(name="wpool", bufs=1))
        qkpool = ctx.enter_context(tc.tile_pool(name="qk", bufs=1))
        vpool = ctx.enter_context(tc.tile_pool(name="vsb", bufs=1))
        mpool = ctx.enter_context(tc.tile_pool(name="msk", bufs=1))
        xpool = ctx.enter_context(tc.tile_pool(name="xin", bufs=1))
        xvpool = ctx.enter_context(tc.tile_pool(name="xvin", bufs=2))
        ptpool = ctx.enter_context(tc.tile_pool(name="ptile", bufs=6))
        npool = ctx.enter_context(tc.tile_pool(name="norm", bufs=1))
        otnpool = ctx.enter_context(tc.tile_pool(name="otn", bufs=1))
        outpool = ctx.enter_context(tc.tile_pool(name="outsb", bufs=2))
        dpool = ctx.enter_context(tc.tile_pool(name="dscr", bufs=2, space="DRAM"))
        # PSUM: 2x2 banks score double-buffer + 2 banks PV accum + 2 banks
        # scratch (projections early / out-proj late) = 8 banks exactly.
        sps_pool = ctx.enter_context(tc.tile_pool(name="sps", bufs=2, space="PSUM"))
        ops_pool = ctx.enter_context(tc.tile_pool(name="ops", bufs=1, space="PSUM"))
        scr_pool = ctx.enter_context(tc.tile_pool(name="scrps", bufs=2, space="PSUM"))

        # ---- persistent SBUF tensors -------------------------------------
        # DMA order: bias rows (tiny, feed the first PE ops), then
        # wq, xq0, wk, xk0, wv, xv0, wo so projections start ~8us in.
        qb_row = wpool.tile([1, DG], bf16)
        kb_row = wpool.tile([1, DG], bf16)
        vb_row = wpool.tile([1, DG], bf16)
        nc.sync.dma_start(qb_row[:], qb.ap())
        nc.sync.dma_start(kb_row[:], kb.ap())
        nc.sync.dma_start(vb_row[:], vb.ap())
        ones_row = wpool.tile([1, P], bf16)
        nc.vector.memset(ones_row[:], 1.0)
        ones2 = wpool.tile([P, 512], bf16)
        nc.vector.memset(ones2[:], 1.0 / P)
        inv128 = wpool.tile([P, P], bf16)
        nc.vector.memset(inv128[:], 1.0 / P)
        qb_sb = wpool.tile([P, DG], bf16)
        kb_sb = wpool.tile([P, DG], bf16)
        vb_sb = wpool.tile([P, DG], bf16)
        for row_, bc_, nm_ in (
            (qb_row, qb_sb, "q"), (kb_row, kb_sb, "k"), (vb_row, vb_sb, "v")
        ):
            bps = scr_pool.tile([P, 512], f32, tag="ps", name=f"bc_{nm_}")
            nc.tensor.matmul(
                bps[:, 0:DG], lhsT=ones_row[:], rhs=row_[:],
                start=True, stop=True,
            )
            nc.scalar.copy(bc_[:], bps[:, 0:DG])
        wq_sb = wpool.tile([P, TDIN, DG], bf16)
        wk_sb = wpool.tile([P, TDIN, DG], bf16)
        wv_sb = wpool.tile([P, TDIN, DG], bf16)
        x0 = []

        def _emit_x_one(xdram, tag, c):
            pool_ = xvpool if tag == "xv" else xpool
            t_ = pool_.tile([P, TDIN, CW], bf16, tag=tag)
            for th_ in range(4):
                nc.sync.dma_start(
                    t_[:, th_ * 2 : (th_ + 1) * 2, :],
                    xdram.ap()[
                        :, th_ * 2 : (th_ + 1) * 2, c * CW : (c + 1) * CW
                    ],
                )
            return t_

        for wsb_, wdr_, xdr_, xtag_ in (
            (wq_sb, wq, xq, "xq"),
            (wk_sb, wk, xk, "xk"),
            (wv_sb, wv, xv, "xv"),
        ):
            for th_ in range(2):
                nc.sync.dma_start(
                    wsb_[:, th_ * 4 : (th_ + 1) * 4, :],
                    wdr_.ap()[:, th_ * 4 : (th_ + 1) * 4, :],
                )
            x0.append(_emit_x_one(xdr_, xtag_, 0))
        wo_sb = wpool.tile([HD, HPG, D], bf16)
        nc.sync.dma_start(wo_sb[:], wo.ap())
        qT_sb = qkpool.tile([P, NT2, S], bf16)   # q projection, transposed
        kT_sb = qkpool.tile([P, NT2, S], bf16)
        # v blocks: per k-tile, per head: [v(64) | ones] -> 65 cols
        v_sb = vpool.tile([P, KT, HPG * (HD + 1)], bf16)
        nc.vector.memset(
            v_sb[:].rearrange("p s (h x) -> p s h x", h=HPG)[:, :, :, HD : HD + 1],
            1.0,
        )
        mask_sb = mpool.tile([P, KT, QW], bf16)
        mk_r = mk.ap().rearrange("(t p) q -> p t q", p=P)

        # warm the exp table set while DMAs stream (off critical path)
        warm = npool.tile([1, 32], bf16, tag="warm")
        nc.vector.memset(warm[:], 0.0)
        nc.scalar.activation(warm[:], warm[:], EXP)

        otn_sb = otnpool.tile([HD, HPG, S], bf16)

        # ---- emit helpers ------------------------------------------------
        def emit_x_dma(c):
            return [
                _emit_x_one(xdram, tag, c)
                for xdram, tag in ((xq, "xq"), (xk, "xk"), (xv, "xv"))
            ]

        def emit_mask_dma(kts, qc):
            for kt in kts:
                nc.sync.dma_start(
                    mask_sb[:, kt, :],
                    mk_r[:, kt, qc * QW : (qc + 1) * QW],
                )

        def emit_qk_tile(c, xch, wsb, bias_sb, dest, half, dt):
            s0 = c * CW + half * 512
            ps = scr_pool.tile([P, 512], f32, tag="ps", name=f"pj{c}{half}{dt}")
            for ktl in range(TDIN):
                nc.tensor.matmul(
                    ps[:],
                    lhsT=wsb[:, ktl, dt * P : (dt + 1) * P],
                    rhs=xch[:, ktl, half * 512 : (half + 1) * 512],
                    start=(ktl == 0),
                    stop=False,
                )
            nc.tensor.matmul(
                ps[:],
                lhsT=bias_sb[:, dt * P : (dt + 1) * P],
                rhs=ones2[:],
                start=False,
                stop=True,
            )
            nc.vector.tensor_copy(dest[:, dt, s0 : s0 + 512], ps[:])

        def emit_v_tile(c, xv_c, m):
            st = c * (CW // P) + m
            ps = scr_pool.tile([P, 512], f32, tag="ps", name=f"pv_{c}_{m}")
            for ktl in range(TDIN):
                nc.tensor.matmul(
                    ps[:, 0:DG],
                    lhsT=xv_c[:, ktl, m * P : (m + 1) * P],
                    rhs=wv_sb[:, ktl, :],
                    start=(ktl == 0),
                    stop=False,
                )
            nc.tensor.matmul(
                ps[:, 0:DG],
                lhsT=inv128[:],
                rhs=vb_sb[:],
                start=False,
                stop=True,
            )
            nc.vector.tensor_copy(
                v_sb[:, st, :].rearrange("p (h x) -> p h x", h=HPG)[:, :, 0:HD],
                ps[:, 0:DG].rearrange("p (h x) -> p h x", h=HPG),
            )

        def emit_proj_chunk(c, xq_c, xk_c, xv_c):
            for xch, wsb, bias_sb, dest in (
                (xq_c, wq_sb, qb_sb, qT_sb),
                (xk_c, wk_sb, kb_sb, kT_sb),
            ):
                for half in range(2):
                    for dt in range(NT2):
                        emit_qk_tile(c, xch, wsb, bias_sb, dest, half, dt)
            for m in range(CW // P):
                emit_v_tile(c, xv_c, m)

        # out-projection for one (st, nch) quarter; 4 accumulating matmuls
        osb2_live = {}

        def emit_outproj_part(st, nch):
            if st not in osb2_live:
                osb2_live[st] = outpool.tile(
                    [P, D], bf16, tag="outsb", name=f"outsb_{st}"
                )
            osb2 = osb2_live[st]
            op_ps = scr_pool.tile([P, 512], f32, tag="ps", name=f"op_{st}_{nch}")
            for h_ in range(HPG):
                nc.tensor.matmul(
                    op_ps[:],
                    lhsT=otn_sb[:, h_, st * P : (st + 1) * P],
                    rhs=wo_sb[:, h_, nch * 512 : (nch + 1) * 512],
                    start=(h_ == 0),
                    stop=(h_ == HPG - 1),
                )
            nc.vector.tensor_copy(osb2[:, nch * 512 : (nch + 1) * 512], op_ps[:])
            if nch == 1:
                nc.sync.dma_start(out.ap()[st * P : (st + 1) * P, :], osb2[:])
                del osb2_live[st]

        # ---- attention ---------------------------------------------------
        def make_head(qc, h):
            """Returns (sc, pv, norm) emitters for one (qc, h)."""
            t, po = h // 2, (h % 2) * HD
            o_ps = ops_pool.tile([HD + 1, QW], f32, tag="ops", name=f"o_{qc}_{h}")
            pts = {}

            def sc(kt, post_mask=None):
                s_ps = sps_pool.tile(
                    [P, QW], f32, tag="sps", name=f"s_{qc}_{h}_{kt}"
                )
                for hf in range(2):
                    nc.tensor.matmul(
                        s_ps[:, hf * 512 : (hf + 1) * 512],
                        lhsT=kT_sb[po : po + HD, t, kt * P : (kt + 1) * P],
                        rhs=qT_sb[
                            po : po + HD,
                            t,
                            qc * QW + hf * 512 : qc * QW + (hf + 1) * 512,
                        ],
                        start=True,
                        stop=True,
                    )
                pt = ptpool.tile([P, QW], bf16, tag="p", name=f"p_{qc}_{h}_{kt}")
                nc.scalar.activation(pt[:], s_ps[:], EXP, scale=0.125)
                nc.vector.tensor_mul(pt[:], pt[:], mask_sb[:, kt, :])
                if post_mask is not None:
                    post_mask(kt)
                pts[kt] = pt

            def pv(kt):
                pt = pts.pop(kt)
                for hf in range(2):
                    nc.tensor.matmul(
                        o_ps[:, hf * 512 : (hf + 1) * 512],
                        lhsT=v_sb[:, kt, h * 65 : (h + 1) * 65],
                        rhs=pt[:, hf * 512 : (hf + 1) * 512],
                        start=(kt == 0),
                        stop=(kt == KT - 1),
                    )

            def norm():
                # baseline normalize: approx-recip of the denominator row,
                # DRAM-bounce broadcast, one TT multiply.
                rec65 = npool.tile([HD + 1, QW], f32, tag="rec")
                nc.vector.reciprocal_approx_fast(out=rec65[:], in_=o_ps[:])
                osb = npool.tile([HD, QW], f32, tag="osb")
                nc.vector.tensor_copy(osb[:], o_ps[0:HD, :])
                scr = dpool.tile([1, QW], f32, tag="scr", name=f"sc_{qc}_{h}")
                nc.sync.dma_start(scr[:], rec65[HD : HD + 1, :])
                rb = npool.tile([HD, QW], f32, tag="rb")
                nc.sync.dma_start(rb[:], scr[:].to_broadcast((HD, QW)))
                nc.vector.tensor_mul(
                    otn_sb[:, h, qc * QW : (qc + 1) * QW], osb[:], rb[:]
                )

            return sc, pv, norm

        # ---- flat scheduler: 8 (qc, h) stages -------------------------
        # Stage i's PV tail + norm are carried into stage i+1's first
        # slots, and stage i pre-emits stage i+1's first two score tiles,
        # so the scalar exp stream never breaks at boundaries. Projection
        # chunk-1 pieces and qc0 out-projections ride along as per-slot
        # PE filler where their dependencies allow.
        def stage_flow(sc, pv, norm, start_kt, queue, post_mask, nxt,
                       min_fill_kt=2):
            state = {"pv": 0}

            def drain_pv(upto):
                while state["pv"] <= min(upto, KT - 3):
                    pv(state["pv"])
                    state["pv"] += 1

            for kt in range(start_kt, KT):
                sc(kt, post_mask)
                if queue and kt >= min_fill_kt:
                    queue.pop(0)()
                if kt >= 5:
                    drain_pv(kt - 2)
                if nxt is not None and kt >= KT - 2:
                    nxt[0](kt - (KT - 2), nxt[1])
            drain_pv(KT - 3)
            return [lambda: pv(KT - 2), lambda: pv(KT - 1), norm]

        # ---- main emission ----------------------------------------------
        emit_mask_dma(range(0, 8), 0)
        xk1 = _emit_x_one(xk, "xk", 1)
        xv1 = _emit_x_one(xv, "xv", 1)
        emit_mask_dma(range(8, KT), 0)
        xq1 = _emit_x_one(xq, "xq", 1)
        emit_proj_chunk(0, *x0)

        stages = [make_head(q_, h_) for q_ in range(QC) for h_ in range(HPG)]
        post_masks = [None] * 8

        def mask_qc1_hook(kt):
            nc.sync.dma_start(mask_sb[:, kt, :], mk_r[:, kt, QW : 2 * QW])

        post_masks[HPG - 1] = mask_qc1_hook

        # filler queues (consumed one per k-tile slot)
        fq = [[] for _ in range(8)]
        # chunk-1 k (dt0 before sc(8)/sc(12) of heads 0/1) and v (before
        # this head's own PVs reach k-tile 8) ride in stage 0
        fq[0] = [
            lambda: emit_qk_tile(1, xk1, wk_sb, kb_sb, kT_sb, 0, 0),
            lambda: emit_qk_tile(1, xk1, wk_sb, kb_sb, kT_sb, 1, 0),
        ] + [(lambda m_=m: emit_v_tile(1, xv1, m_)) for m in range(8)]
        # k dt1 (heads 2/3) + all of q chunk 1 (needed at qc1) in stage 1
        fq[1] = [
            lambda: emit_qk_tile(1, xk1, wk_sb, kb_sb, kT_sb, 0, 1),
            lambda: emit_qk_tile(1, xk1, wk_sb, kb_sb, kT_sb, 1, 1),
            lambda: emit_qk_tile(1, xq1, wq_sb, qb_sb, qT_sb, 0, 0),
            lambda: emit_qk_tile(1, xq1, wq_sb, qb_sb, qT_sb, 1, 0),
        ]
        fq[2] = [lambda: emit_qk_tile(1, xq1, wq_sb, qb_sb, qT_sb, 0, 1)]
        fq[3] = [lambda: emit_qk_tile(1, xq1, wq_sb, qb_sb, qT_sb, 1, 1)]
        # qc0 out-projection fillers during qc1 stages
        opj = [(st, nch) for st in range(S // P // 2) for nch in range(2)]
        for h in range(HPG):
            fq[4 + h] = [
                (lambda s=st, n=nch: emit_outproj_part(s, n))
                for st, nch in opj[h * 4 : (h + 1) * 4]
            ]

        carry = []
        for i in range(8):
            sc, pv, norm = stages[i]
            nxt = (stages[i + 1][0], post_masks[i + 1]) if i < 7 else None
            carry = stage_flow(
                sc, pv, norm,
                start_kt=0 if i == 0 else 2,
                queue=carry + fq[i],
                post_mask=post_masks[i],
                nxt=nxt,
                min_fill_kt=4 if i == 0 else 2,
            )
        for fn in carry:
            fn()

        # qc1 out-proj tail: two PSUM pipelines (scratch + freed score
        # banks) so the 16 remaining parts drain twice as wide.
        def emit_outproj_sps(st):
            op = sps_pool.tile([P, QW], f32, tag="sps", name=f"opx_{st}")
            for nch in range(2):
                for h_ in range(HPG):
                    nc.tensor.matmul(
                        op[:, nch * 512 : (nch + 1) * 512],
                        lhsT=otn_sb[:, h_, st * P : (st + 1) * P],
                        rhs=wo_sb[:, h_, nch * 512 : (nch + 1) * 512],
                        start=(h_ == 0),
                        stop=(h_ == HPG - 1),
                    )
            osb2 = outpool.tile([P, D], bf16, tag="outsb", name=f"ox_{st}")
            nc.vector.tensor_copy(osb2[:], op[:])
            nc.sync.dma_start(out.ap()[st * P : (st + 1) * P, :], osb2[:])

        for st in range(S // P // 2, S // P):
            if st % 2 == 0:
                emit_outproj_sps(st)
            else:
                emit_outproj_part(st, 0)
                emit_outproj_part(st, 1)

    nc.compile()
    return nc


@functools.lru_cache(maxsize=1)
def _graph():
    return build_graph()


def make_in_maps(
    query, key, value, mask,
    wq_kernel, wq_bias, wk_kernel, wk_bias,
    wv_kernel, wv_bias, wo_kernel, wo_bias,
):
    q = np.asarray(query, np.float32)
    k = np.asarray(key, np.float32)
    v = np.asarray(value, np.float32)
    mask = np.asarray(mask)
    wqk = np.asarray(wq_kernel, np.float32)
    wkk = np.asarray(wk_kernel, np.float32)
    wvk = np.asarray(wv_kernel, np.float32)
    wok = np.asarray(wo_kernel, np.float32)

    def tile_x(a):  # [S, D] -> [P, TDIN, S] pre-tiled transpose
        return np.ascontiguousarray(
            a.T.reshape(TDIN, P, S).transpose(1, 0, 2)
        ).astype(BF16)

    xt = [[tile_x(x[b]) for x in (q, k, v)] for b in range(B)]
    mt = [
        np.ascontiguousarray(mask[b].T.astype(np.float32)).astype(BF16)
        for b in range(B)
    ]
    in_maps = []
    for c in range(NCORES):
        b, g = divmod(c, GH)
        cs = slice(g * DG, (g + 1) * DG)
        wo_arr = np.ascontiguousarray(
            wok[cs, :].reshape(HPG, HD, D).transpose(1, 0, 2)
        ).astype(BF16)
        in_maps.append(
            {
                "xq_t": xt[b][0],
                "xk_t": xt[b][1],
                "xv_t": xt[b][2],
                "mask_t": mt[b],
                "wq": np.ascontiguousarray(wqk[:, cs].reshape(TDIN, P, DG).transpose(1, 0, 2)).astype(BF16),
                "wk": np.ascontiguousarray(wkk[:, cs].reshape(TDIN, P, DG).transpose(1, 0, 2)).astype(BF16),
                "wv": np.ascontiguousarray(wvk[:, cs].reshape(TDIN, P, DG).transpose(1, 0, 2)).astype(BF16),
                "wo": wo_arr,
                "qb": np.asarray(wq_bias, np.float32)[cs].reshape(1, DG).astype(BF16),
                "kb": np.asarray(wk_bias, np.float32)[cs].reshape(1, DG).astype(BF16),
                "vb": np.asarray(wv_bias, np.float32)[cs].reshape(1, DG).astype(BF16),
            }
        )
    return in_maps


def combine_outputs(results, wo_bias):
    outs = np.stack([np.asarray(r["out"], np.float32) for r in results])
    full = outs.reshape(B, GH, S, D).sum(axis=1)
    return (full + np.asarray(wo_bias, np.float32)[None, None, :]).astype(
        np.float32
    )


def kernel(**inputs):
    from concourse import bass_utils

    nc = _graph()
    in_maps = make_in_maps(**inputs)
    res = bass_utils.run_bass_kernel_spmd(
        nc, in_maps, core_ids=list(range(NCORES))
    )
    return combine_outputs(res.results, inputs["wo_bias"])
